# revision 40
# baseline (speedup 1.0000x reference)
"""AttentionAggregator (GAT-style) Trainium2 Bass kernel — v12.

Math (per reference):
    xw  = x @ W                                   [N, 128]
    xn  = xw[neibs]                               [N, 32, 128]
    e   = leakyrelu(xw@a_self + xn@a_neib, 0.2)   [N, 32]
    att = softmax(e, axis=1)
    h'  = sum_s att[:,s] * xn[:,s,:]              [N, 128]
    out = elu(concat([xw, h'], 1))                [N, 256]

v12 (2.54ms -> ~0.75ms): the per-slot SWDGE indirect DMAs (49x33 per
core, ~1us fixed emission each — the v3 bottleneck) are replaced by TWO
dma_gathers per 128-node tile (2176 int16 indices each), rotated across
the 4 SWDGE queues so emissions overlap (Q7 core pairs work
independently) and each instruction fills only half a queue's
descriptor ring, so the decode's await_space no longer head-of-line
blocks the Pool stream.  Phase-2 scalar math is batched once per 4-tile
group on [P, nb, *] views (DVE instruction dispatch ~1us contended
dominates small ops); gather idx streams are DMA-streamed per group.
The int16 index range covers the 50176-row table because the Q7 address
math (IVP_MULUSAN_2X32) multiplies stride x SIGNED index: with the
gather base at row 17408, idx = row - 17408 spans the whole table.
Half-gather A ends in each node's LARGEST neighbor row (>= 17408 w.p.
1-3e-15) and half B in the pad slot, so the Q7's trailing-negative-idx
trim never fires.  num_idxs>1024 requires single_packet=False (the
one-packet framing caps at 64 descriptors per SDMA engine).

  * a_neib is folded into W on the host: table rows store xw' = xw*a_neib
    (bf16, 256B rows) with the d* column (d* = argmax|a_neib|, permuted
    last) replaced by an fp16 value pre = xw'_d* + sum(other cols).  The
    attention logit needs only that scalar (col 127); xw'_d* is
    reconstructed as pre - rowsum.  h' divides by a_neib at the end.
  * Self row is gathered as slot 0; slot 33 is index-stream tail padding
    (trailing negative int16 idxs would be trimmed by the Q7).
  * Phase 1 runs in bf16: x.T and W*a_neib are host-prepared bf16; one
    128-col matmul pair per 128 nodes.
"""

import sys

for _p in ("/opt/trn_rl_repo",):
    if _p not in sys.path:
        sys.path.insert(0, _p)

import numpy as np
import ml_dtypes

BF16 = ml_dtypes.bfloat16

N = 50000
S = 32
D_IN = 256
D_OUT = 128
ALPHA = 0.2

USE_F16PRE = True     # store pre col as fp16 (else bf16)
USE_ACC = True        # use activation(accum_out=)

NCORES = 8
SHARD = N // NCORES             # 6250
SHARD_PAD = 6272                # 49 * 128
NPAD = NCORES * SHARD_PAD       # 50176
P = 128
SG = S + 1                      # 33 live slots: self + 32 neighbors
SGP = SG + 1                    # 34 slots incl index-stream pad
GBASE = 17408                   # dma_gather base row (signed idx16 reach)
NH = (SGP * P) // 2             # 2176 idxs per half-gather
NIDX = SGP * P                  # 4352 idxs per tile
IDXW = NIDX // 16               # 272 int16 cols per tile in the idx buffer

# phase-1 blocks: (row_start, n_rows, rows_per_partition)
P1_BLOCKS = []
_s = 0
while _s < NPAD:
    _nt = min(2048, NPAD - _s)
    P1_BLOCKS.append((_s, _nt, _nt // P))
    _s += _nt

# phase-2 out blocks per core: (out_row_start, n_tiles_of_128)
P2_BLOCKS = []
_o = 0
while _o < SHARD_PAD:
    _nb = min(4, (SHARD_PAD - _o) // P)
    P2_BLOCKS.append((_o, _nb))
    _o += _nb * P
NTILES = sum(nb for _, nb in P2_BLOCKS)   # 49

_prog_cache = {}


def interleave_rows(f):
    """Flat index (core*SHARD_PAD + rank) -> table row with the per-block
    (partition, chunk) transpose that makes phase-1 table writes contiguous
    per partition."""
    f = np.asarray(f)
    t = np.minimum(f // 2048, len(P1_BLOCKS) - 1)
    base = t * 2048
    kb = np.where(base >= P1_BLOCKS[-1][0], P1_BLOCKS[-1][2], 16)
    w = f - base
    k = w // P
    p = w % P
    return base + p * kb + k


def wrap_idx16(vals):
    """Consumed-position i -> idx buffer (partition i%16, col i//16),
    replicated across the eight 16-partition Q7 groups."""
    n = vals.shape[-1]
    buf = np.zeros((vals.shape[0], 16, n // 16), np.int16)
    pos = np.arange(n)
    buf[:, pos % 16, pos // 16] = vals
    return np.tile(buf, (1, 8, 1))


def prep_inputs(x, neibs, W, a):
    x = np.asarray(x, np.float32)
    neibs = np.asarray(neibs).astype(np.int64)
    W = np.asarray(W, np.float32)
    a = np.asarray(a, np.float32).reshape(-1)
    a_self, a_neib = a[:D_OUT], a[D_OUT:]

    # column permutation: d* (max |a_neib|) last
    dstar = int(np.argmax(np.abs(a_neib)))
    cp = np.concatenate([np.delete(np.arange(D_OUT), dstar), [dstar]])

    Wn = W * a_neib[None, :]
    rhsm = np.empty((D_IN, D_OUT), np.float32)
    rhsm[:, :127] = Wn[:, cp[:127]]
    rhsm[:, 127] = Wn[:, dstar]
    rhsm = rhsm.astype(BF16)

    # packed per-partition constants (f32), layout:
    #   [0:508)     ratd = (ratq[c] - ratq[127]) for c<127, tiled x4
    #   [508:1016)  recq[0:127] tiled x4
    #   [1016]      ratq[127]   [1017] recq[127]
    #   [1018:1026) ones (identity operand for fused tail scales)
    ratq_v = (a_self / a_neib)[cp]
    recq_v = (1.0 / a_neib)[cp]
    cq = np.ones((P, 1026), np.float32)
    cq[:, 0:508] = np.tile(ratq_v[:127] - ratq_v[127], 4)[None, :]
    cq[:, 508:1016] = np.tile(recq_v[:127], 4)[None, :]
    cq[:, 1016] = ratq_v[127]
    cq[:, 1017] = recq_v[127]

    # node -> table row (identity rank order, interleaved within blocks)
    f_node = np.arange(N) // SHARD * SHARD_PAD + np.arange(N) % SHARD
    row_node = interleave_rows(f_node)

    # x.T in flat (core, rank) order
    xt = np.zeros((D_IN, NPAD), BF16)
    xt[:, f_node] = x.T.astype(BF16)

    # per-core gather idx16: tile ti position s*128+p = slot s of node p
    in_maps = []
    for c in range(NCORES):
        rows = np.zeros((NTILES, NIDX), np.int64)
        ti = 0
        for (ostart, nb) in P2_BLOCKS:
            for b in range(nb):
                o = ostart + np.arange(P) * nb + b        # out rows = ranks
                real = o < SHARD
                noden = np.where(real, c * SHARD + o, 0)
                blk = np.empty((SG, P), np.int64)
                blk[0, :] = np.where(real, row_node[noden], 0)
                # sorted rows per node (order is softmax-invariant); the
                # LARGEST goes to slot 16 = the tail of half-gather A, so
                # its idx16 is non-negative (no trailing trim; P(all 32
                # rows < GBASE) ~ 3e-15)
                srt = np.where(real[:, None],
                               np.sort(row_node[neibs[noden]], axis=1),
                               GBASE)
                blk[1:16, :] = srt[:, 0:15].T
                blk[16, :] = srt[:, 31]
                blk[17:, :] = srt[:, 15:31].T
                rows[ti, :SG * P] = blk.reshape(-1)
                rows[ti, SG * P:] = GBASE          # pad tail: idx 0
                ti += 1
        r16 = (rows - GBASE).astype(np.int16)
        ia = wrap_idx16(r16[:, :NH])                  # [NTILES, 128, 136]
        ib = wrap_idx16(r16[:, NH:])
        idx16 = np.concatenate([ia, ib], axis=2)      # [NTILES, 128, 272]
        idx16 = idx16.transpose(1, 0, 2).reshape(P, NTILES * IDXW).copy()
        in_maps.append({
            "xt": xt, "rhsm": rhsm, "cq": cq, "idx16": idx16,
        })

    post = dict(cp=cp, bounds=None)
    return in_maps, post


def build_program(bounds=None, debug=False):
    import concourse.bass as bass
    import concourse.bacc as bacc
    import concourse.tile as tile
    from concourse import mybir
    from concourse.bass import _add_dep_helper

    f32 = mybir.dt.float32
    bf16 = mybir.dt.bfloat16
    f16 = mybir.dt.float16
    i16 = mybir.dt.int16
    AL = mybir.AluOpType
    AF = mybir.ActivationFunctionType

    nc = bacc.Bacc("TRN2", target_bir_lowering=False, debug=False,
                   num_devices=NCORES, num_swdge_queues=4)

    xt = nc.dram_tensor("xt", [D_IN, NPAD], bf16, kind="ExternalInput").ap()
    rhsm = nc.dram_tensor("rhsm", [D_IN, D_OUT], bf16,
                          kind="ExternalInput").ap()
    cq = nc.dram_tensor("cq", [P, 1026], f32, kind="ExternalInput").ap()
    idx16 = nc.dram_tensor("idx16", [P, NTILES * IDXW], i16,
                           kind="ExternalInput").ap()
    out = nc.dram_tensor("out", [SHARD_PAD, 2 * D_OUT], f32,
                         kind="ExternalOutput").ap()

    with tile.TileContext(nc) as tc:
        const_pool = tc.alloc_tile_pool(name="const", bufs=1)
        dram_pool = tc.alloc_tile_pool(name="dram", bufs=1, space="DRAM")
        p1_pool = tc.alloc_tile_pool(name="p1", bufs=2)
        psum_pool = tc.alloc_tile_pool(name="psum", bufs=3, space="PSUM")
        g_pool = tc.alloc_tile_pool(name="g", bufs=3)
        prod_pool = tc.alloc_tile_pool(name="prod", bufs=1)
        sm_pool = tc.alloc_tile_pool(name="sm", bufs=2)
        vg_pool = tc.alloc_tile_pool(name="vg", bufs=2)
        em_pool = tc.alloc_tile_pool(name="em", bufs=1)

        table = dram_pool.tile([NPAD, D_OUT], bf16)

        rhs0 = const_pool.tile([P, D_OUT], bf16)
        rhs1 = const_pool.tile([P, D_OUT], bf16)
        nc.sync.dma_start(out=rhs0[:, :], in_=rhsm[0:P, :])
        nc.sync.dma_start(out=rhs1[:, :], in_=rhsm[P:D_IN, :])
        cq_t = const_pool.tile([P, 1026], f32)
        nc.sync.dma_start(out=cq_t[:, :], in_=cq[:, :])
        idx_pool = tc.alloc_tile_pool(name="idx", bufs=3)

        # ---- phase 1: projected table (replicated on all cores) ----------
        p1_writes = []
        for (r0, nt, kb) in P1_BLOCKS:
            xt0 = p1_pool.tile([P, nt], bf16, tag="xt0")
            xt1 = p1_pool.tile([P, nt], bf16, tag="xt1")
            nc.sync.dma_start(out=xt0[:, :], in_=xt[0:P, r0:r0 + nt])
            nc.sync.dma_start(out=xt1[:, :], in_=xt[P:D_IN, r0:r0 + nt])
            tt = p1_pool.tile([P, nt], bf16, tag="tt")
            ttv = tt[:, :].rearrange("p (k c) -> p k c", c=D_OUT)
            dstash = p1_pool.tile([P, kb], f32, tag="dstash")
            for kk in range(nt // 512):
                ps = psum_pool.tile([P, 512], f32, tag="mm")
                for j in range(4):
                    k = kk * 4 + j
                    sl = slice(k * P, (k + 1) * P)
                    pj = slice(j * P, (j + 1) * P)
                    nc.tensor.matmul(ps[:, pj], lhsT=xt0[:, sl],
                                     rhs=rhs0[:, :], start=True, stop=False)
                    nc.tensor.matmul(ps[:, pj], lhsT=xt1[:, sl],
                                     rhs=rhs1[:, :], start=False, stop=True)
                nc.scalar.copy(out=tt[:, kk * 512:(kk + 1) * 512],
                               in_=ps[:, :])
                psv = ps[:, :].rearrange("p (j c) -> p j c", c=P)
                nc.vector.tensor_copy(
                    dstash[:, kk * 4:(kk + 1) * 4],
                    psv[:, :, 127:128].rearrange("p j c -> p (j c)"))
            # col 127 := fp16(xw'_d* + sum of the 127 stored bf16 cols);
            # phase 2 reconstructs xw'_d* = col127 - rowsum exactly.
            rs = p1_pool.tile([P, kb], f32, tag="rs")
            nc.vector.tensor_reduce(out=rs[:, :], in_=ttv[:, :, 0:127],
                                    axis=mybir.AxisListType.X, op=AL.add)
            if USE_F16PRE:
                tt16 = tt[:, :].bitcast(f16).rearrange(
                    "p (k c) -> p k c", c=D_OUT)
            else:
                tt16 = ttv
            nc.vector.tensor_tensor(
                out=tt16[:, :, 127:128].rearrange("p k c -> p (k c)"),
                in0=dstash[:, :], in1=rs[:, :], op=AL.add)
            w = nc.sync.dma_start(
                out=table[r0:r0 + nt, :].rearrange("(p k) c -> p k c", k=kb),
                in_=tt[:, :].rearrange("p (k c) -> p k c", c=D_OUT))
            p1_writes.append(w)

        # ---- phase 2: gather + attention (sharded), group-batched --------
        # All per-node scalar math runs once per nb-tile group on [P, nb, *]
        # views (DVE instruction dispatch ~1us contended dominates small ops)
        ti = 0
        for (ostart, nb) in P2_BLOCKS:
            vg = vg_pool.tile([P, nb * 2 * D_OUT], f32, tag="vg")
            vgv = vg[:, :].rearrange("p (k c) -> p k c", c=2 * D_OUT)
            idxg = idx_pool.tile([P, nb * IDXW], i16, tag="idxg")
            nc.sync.dma_start(
                out=idxg[:, :],
                in_=idx16[:, ti * IDXW:(ti + nb) * IDXW])
            g4 = g_pool.tile([P, nb * SGP * D_OUT], bf16, tag="g4")
            g4v = g4[:, :].rearrange("p (g s c) -> p g s c", s=SGP, c=D_OUT)
            g416 = g4[:, :].bitcast(f16).rearrange(
                "p (g s c) -> p g s c", s=SGP, c=D_OUT)
            # two half-gathers per tile (slots 0..16 / 17..33): 137
            # descs/engine each, so a queue's ring holds two and the
            # decode's await_space stops head-of-line-blocking the stream
            for b in range(nb):
                for h in range(2):
                    gi = nc.gpsimd.dma_gather(
                        g4v[:, b, 17 * h:17 * (h + 1), :],
                        table[GBASE:NPAD, :],
                        idxg[:, b * IDXW + h * (IDXW // 2):
                             b * IDXW + (h + 1) * (IDXW // 2)],
                        NH, NH, D_OUT, single_packet=False,
                        queue_num=(2 * (ti + b) + h) % 4)
                    # negative idxs read table rows < GBASE: order after
                    # all phase-1 writes
                    for w in p1_writes:
                        _add_dep_helper(gi.ins, w.ins, sync=True,
                                        reason="gather reads full table")
            ti += nb

            selfcols = g4v[:, :, 0, 0:127]                  # [P, nb, 127]
            selfpre = g416[:, :, 0, 127:128].rearrange(
                "p g c -> p (g c)")                         # [P, nb] f16
            ratd4 = cq_t[:, 0:nb * 127].rearrange(
                "p (g c) -> p g c", c=127)
            recq4 = cq_t[:, 508:508 + nb * 127].rearrange(
                "p (g c) -> p g c", c=127)

            jj = sm_pool.tile([P, nb * 127], bf16, tag="jj")
            nc.vector.tensor_tensor(
                out=jj[:, :].rearrange("p (g c) -> p g c", c=127),
                in0=selfcols, in1=ratd4, op=AL.mult)
            s1 = sm_pool.tile([P, nb], f32, tag="s1")
            nc.vector.tensor_reduce(
                out=s1[:, :],
                in_=jj[:, :].rearrange("p (g c) -> p g c", c=127),
                axis=mybir.AxisListType.X, op=AL.add)
            rowsum = sm_pool.tile([P, nb], f32, tag="rowsum")
            nc.vector.tensor_reduce(out=rowsum[:, :], in_=selfcols,
                                    axis=mybir.AxisListType.X, op=AL.add)
            tails = sm_pool.tile([P, nb * 2], f32, tag="tails")
            tailsv = tails[:, :].rearrange("p (g u) -> p g u", u=2)
            xwds = tailsv[:, :, 0:1].rearrange("p g u -> p (g u)")
            nc.vector.tensor_tensor(out=xwds, in0=selfpre,
                                    in1=rowsum[:, :], op=AL.subtract)
            # pre_s = selfpre*rat127 + sum(selfrow[0:127]*ratd)
            pre_s = sm_pool.tile([P, nb], f32, tag="pre_s")
            nc.vector.scalar_tensor_tensor(
                out=pre_s[:, :], in0=selfpre, scalar=cq_t[:, 1016:1017],
                in1=s1[:, :], op0=AL.mult, op1=AL.add)

            # logits: gathered fp16 col 127 of slots 1..32, + pre_s, lrelu
            e_pre = g416[:, :, 1:SG, 127:128].rearrange(
                "p g s c -> p g (s c)")                     # [P, nb, 32]
            eraw = sm_pool.tile([P, nb * S], f32, tag="eraw")
            erawv = eraw[:, :].rearrange("p (g s) -> p g s", s=S)
            nc.vector.tensor_tensor(
                out=erawv, in0=e_pre,
                in1=pre_s[:, :].to_broadcast([P, nb, S]), op=AL.add)
            e = sm_pool.tile([P, nb * S], f32, tag="e")
            nc.vector.scalar_tensor_tensor(
                out=e[:, :], in0=eraw[:, :], scalar=ALPHA,
                in1=eraw[:, :], op0=AL.mult, op1=AL.max)
            # softmax shift: one shared max over the whole group (softmax is
            # shift-invariant per row; group max >= tile max keeps exp<=1)
            negm = sm_pool.tile([P, 1], f32, tag="negm")
            nc.vector.tensor_reduce(out=negm[:, :], in_=e[:, :],
                                    axis=mybir.AxisListType.X,
                                    op=AL.max, negate=True)
            p_un = sm_pool.tile([P, nb * S], bf16, tag="p_un")
            nc.scalar.activation(p_un[:, :], e[:, :], AF.Exp,
                                 bias=negm[:, :], scale=1.0)
            p_unv = p_un[:, :].rearrange("p (g s) -> p g s", s=S)
            ssum = sm_pool.tile([P, nb], f32, tag="ssum")
            nc.vector.tensor_reduce(out=ssum[:, :], in_=p_unv,
                                    axis=mybir.AxisListType.X, op=AL.add)
            rinv = sm_pool.tile([P, nb], f32, tag="rinv")
            nc.vector.reciprocal(rinv[:, :], ssum[:, :])

            # weighted neighbor sum (normalized in the product; per tile —
            # the stt scalar must be [P,1])
            prod4 = prod_pool.tile([P, nb * S * D_OUT], f16, tag="prod4")
            prod4v = prod4[:, :].rearrange(
                "p (g s d) -> p g s d", s=S, d=D_OUT)
            for b in range(nb):
                nc.vector.scalar_tensor_tensor(
                    out=prod4v[:, b, :, :], in0=g4v[:, b, 1:SG, :],
                    scalar=rinv[:, b:b + 1],
                    in1=p_unv[:, b, :].to_broadcast([P, S, D_OUT]),
                    op0=AL.mult, op1=AL.mult)
            # batched binary-tree sum over slots (4D APs)
            t2 = prod_pool.tile([P, nb * (S // 2) * D_OUT], f16, tag="t2")
            t2v = t2[:, :].rearrange(
                "p (g s d) -> p g s d", s=S // 2, d=D_OUT)
            nc.vector.tensor_tensor(
                out=t2v[:, :, :, :], in0=prod4v[:, :, 0:S // 2, :],
                in1=prod4v[:, :, S // 2:S, :], op=AL.add)
            kslots = S // 2
            while kslots > 2:
                h = kslots // 2
                nc.vector.tensor_tensor(
                    out=t2v[:, :, 0:h, :], in0=t2v[:, :, 0:h, :],
                    in1=t2v[:, :, h:2 * h, :], op=AL.add)
                kslots = h
            hh = sm_pool.tile([P, nb * D_OUT], f32, tag="hh")
            hhv = hh[:, :].rearrange("p (g d) -> p g d", d=D_OUT)
            nc.vector.tensor_tensor(
                out=hhv, in0=t2v[:, :, 0, :], in1=t2v[:, :, 1, :],
                op=AL.add)
            hsum = sm_pool.tile([P, nb], f32, tag="hsum")
            nc.vector.tensor_reduce(out=hsum[:, :], in_=hhv[:, :, 0:127],
                                    axis=mybir.AxisListType.X, op=AL.add)
            # h'_d* = (sum_s att*pre) - sum_{d<127} h'_d;
            # epre = eraw - pre_s (exact to 1 ulp)
            epre = sm_pool.tile([P, nb * S], f32, tag="epre")
            nc.vector.tensor_tensor(
                out=epre[:, :].rearrange("p (g s) -> p g s", s=S),
                in0=erawv, in1=pre_s[:, :].to_broadcast([P, nb, S]),
                op=AL.subtract)
            junk2 = sm_pool.tile([P, nb * S], bf16, tag="junk2")
            nc.vector.tensor_tensor(out=junk2[:, :], in0=epre[:, :],
                                    in1=p_un[:, :], op=AL.mult)
            attdot = sm_pool.tile([P, nb], f32, tag="attdot")
            nc.vector.tensor_reduce(
                out=attdot[:, :],
                in_=junk2[:, :].rearrange("p (g s) -> p g s", s=S),
                axis=mybir.AxisListType.X, op=AL.add)
            t1 = sm_pool.tile([P, nb], f32, tag="t1")
            nc.vector.tensor_tensor(out=t1[:, :], in0=attdot[:, :],
                                    in1=rinv[:, :], op=AL.mult)
            hds = tailsv[:, :, 1:2].rearrange("p g u -> p (g u)")
            nc.vector.tensor_tensor(out=hds, in0=t1[:, :],
                                    in1=hsum[:, :], op=AL.subtract)

            # assemble output rows [xw, h'] (device column order)
            nc.vector.tensor_tensor(out=vgv[:, :, 0:127],
                                    in0=selfcols, in1=recq4, op=AL.mult)
            nc.vector.tensor_tensor(out=vgv[:, :, 128:255],
                                    in0=hhv[:, :, 0:127], in1=recq4,
                                    op=AL.mult)
            # cols 127/255 of each row: (tails * recq127) * 1.0
            vtails = vgv[:, :, :].rearrange(
                "p g (u v) -> p g u v", v=D_OUT)[:, :, :, 127:128].rearrange(
                "p g u v -> p (g u v)")
            nc.vector.scalar_tensor_tensor(
                out=vtails, in0=tails[:, :], scalar=cq_t[:, 1017:1018],
                in1=cq_t[:, 1018:1018 + 2 * nb], op0=AL.mult, op1=AL.mult)

            # elu, batched over the whole group's output rows
            em = em_pool.tile([P, nb * 2 * D_OUT], f32, tag="em")
            nc.scalar.activation(em[:, :], vg[:, :], AF.Exp)
            nc.vector.tensor_scalar(out=em[:, :], in0=em[:, :],
                                    scalar1=-1.0, scalar2=0.0,
                                    op0=AL.add, op1=AL.min)
            nc.vector.scalar_tensor_tensor(
                out=vg[:, :], in0=vg[:, :], scalar=0.0,
                in1=em[:, :], op0=AL.max, op1=AL.add)

            nc.sync.dma_start(
                out=out[ostart:ostart + nb * P, :].rearrange(
                    "(p k) c -> p k c", k=nb),
                in_=vg[:, :].rearrange("p (k c) -> p k c", c=2 * D_OUT))

        for _pool in (idx_pool, em_pool, vg_pool, sm_pool, prod_pool, g_pool,
                      psum_pool, p1_pool, dram_pool, const_pool):
            _pool.release()

    nc.compile()
    return nc


def run_spmd(nc, in_maps, trace=False):
    from concourse import bass_utils
    res = bass_utils.run_bass_kernel_spmd(
        nc, in_maps, core_ids=list(range(NCORES)), trace=trace)
    return res


def postprocess(res_list, post):
    cp = post["cp"]
    colmap = np.concatenate([cp, D_OUT + cp])
    full = np.empty((N, 2 * D_OUT), np.float32)
    for c in range(NCORES):
        dev = np.asarray(res_list[c])
        full[c * SHARD:(c + 1) * SHARD, colmap] = dev[:SHARD]
    return full


def kernel(x, neibs, W, a):
    in_maps, post = prep_inputs(x, neibs, W, a)
    if "prog" not in _prog_cache:
        _prog_cache["prog"] = build_program()
    nc = _prog_cache["prog"]
    res = run_spmd(nc, in_maps)
    outs = [np.asarray(res.results[c]["out"]) for c in range(NCORES)]
    return postprocess(outs, post).astype(np.float32)


if __name__ == "__main__":
    print("module ok")
